# revision 1
# baseline (speedup 1.0000x reference)
"""Trainium2 Bass kernel for nn_BaselineOut (article/option additive-attention MRC head).

Contract: kernel(**inputs) takes FULL unsharded inputs (numpy), returns FULL
[32, 5] float32 logits.  Internally: data-parallel over batch across 8 cores
(4 batch items per core), all params replicated.

Math notes (vs reference):
  - oqc gather is done as a one-hot matmul on device (host only encodes the
    int indices as a one-hot fp32 matrix - a layout/encoding transform).
  - V-projection is pulled out of the attention sum by linearity:
        sum_l softmax_l * (V @ Vw^T + Vb) = (sum_l softmax_l * V) @ Vw^T + Vb
    so the [B*L,H]x[H,H] V matmul collapses to a weighted sum over L plus a
    tiny [B,H]x[H,H] matmul.
  - Consecutive linear maps with no nonlinearity between are constant-folded
    on host (weight-weight products):
      * aq -> Qp_d: one matmul with Wqv = d_Qw @ a_Vw^T and a folded bias.
      * feats -> logits: per-option folded weights Ff_o = d_Vw^T @ f_w[:,o]^T.
  - softmax logit bias (vb) is dropped: softmax is shift-invariant.
  - exp is computed without max-subtraction: |logit| <= ||vw||_1 ~ 36, well
    inside fp32 exp range.
  - Large matmuls run with float32r operands (full-rate fp32 on the PE);
    f32r must never be a DRAM I/O dtype (crashes NRT) - the f32->f32r cast
    happens in SWDGE DMAs.
"""

import functools
import sys

import numpy as np

sys.path.insert(0, "/opt/trn_rl_repo")

import concourse.bass as bass  # noqa: E402
from concourse import bacc  # noqa: E402
import concourse.tile as tile  # noqa: E402
from concourse import mybir  # noqa: E402
from concourse.bass import ds, ts  # noqa: E402

B, LA, LQ, LO, H, OUT = 32, 2048, 64, 32, 1024, 5
NCORES = 8
BL = B // NCORES  # 4 batch items per core
NOPT = 5
F32 = mybir.dt.float32
F32R = mybir.dt.float32r
LT = 512  # article l-tile (free dim of the big matmuls)
NLT = LA // LT  # 4
C = H // 128  # 8 h-chunks
BO = BL * NOPT  # 20 (b, option) pairs per core
AF = mybir.ActivationFunctionType
ALU = mybir.AluOpType
AX = mybir.AxisListType
OUTP = 8  # final-linear out dim padded even for f32r


def build_nc() -> bass.Bass:
    nc = bacc.Bacc("TRN2", target_bir_lowering=False, debug=False)

    # ---- DRAM I/O (per-core shard; names are the in_map keys) ----
    artT = nc.dram_tensor("artT", [BL, H, LA], F32, kind="ExternalInput").ap()
    optT = nc.dram_tensor("optT", [BL, H, NOPT, LO], F32, kind="ExternalInput").ap()
    qcd = nc.dram_tensor("qc", [BL, LQ, H], F32, kind="ExternalInput").ap()
    ohd = nc.dram_tensor("oh", [LQ, BL], F32, kind="ExternalInput").ap()
    wQa = nc.dram_tensor("aQwT", [H, H], F32, kind="ExternalInput").ap()
    wKa = nc.dram_tensor("aKwT", [H, H], F32, kind="ExternalInput").ap()
    wQV = nc.dram_tensor("qvwT", [H, H], F32, kind="ExternalInput").ap()
    wKd = nc.dram_tensor("dKwT", [H, H], F32, kind="ExternalInput").ap()
    vwad = nc.dram_tensor("vwaT", [128, C], F32, kind="ExternalInput").ap()
    vwdd = nc.dram_tensor("vwdT", [128, C], F32, kind="ExternalInput").ap()
    qkbd = nc.dram_tensor("qkbT", [128, C], F32, kind="ExternalInput").ap()
    qvbd = nc.dram_tensor("qvbT", [128, C], F32, kind="ExternalInput").ap()
    fwd = nc.dram_tensor("fwT", [128, NOPT, C, OUTP], F32, kind="ExternalInput").ap()
    fbd = nc.dram_tensor("fb", [BL, OUTP], F32, kind="ExternalInput").ap()
    onesd = nc.dram_tensor("ones1", [1, 128], F32, kind="ExternalInput").ap()
    outd = nc.dram_tensor("out", [BL, OUT], F32, kind="ExternalOutput").ap()

    with (
        tile.TileContext(nc) as tc,
        nc.allow_low_precision(reason="float32r is 4-byte; PE accumulates fp32"),
    ):
        with (
            tc.tile_pool(name="stream", bufs=3) as stream,
            tc.tile_pool(name="wbig", bufs=3) as wbig,
            tc.tile_pool(name="mpool", bufs=3) as mpool,
            tc.tile_pool(name="spool", bufs=2) as spool,
            tc.tile_pool(name="rpool", bufs=2) as rpool,
            tc.tile_pool(name="rdpool", bufs=1) as rdpool,
            tc.tile_pool(name="ubuf", bufs=2) as ubuf,
            tc.tile_pool(name="scratch", bufs=1) as scratch,
            tc.tile_pool(name="one", bufs=1) as one,
            tc.tile_pool(name="pacc", bufs=4, space="PSUM") as pacc,
            tc.tile_pool(name="prow", bufs=2, space="PSUM") as prow,
            tc.tile_pool(name="psml", bufs=2, space="PSUM") as psml,
        ):
            # ---------- small constant loads (ahead of big weights) ----------
            vwa = one.tile([128, C], F32R, tag="vwa")
            nc.gpsimd.dma_start(out=vwa, in_=vwad)
            vwd = one.tile([128, C], F32R, tag="vwd")
            nc.gpsimd.dma_start(out=vwd, in_=vwdd)
            ones = one.tile([1, 128], F32R, tag="ones")
            nc.gpsimd.dma_start(out=ones, in_=onesd)
            qkb = one.tile([128, C], F32, tag="qkb")
            nc.sync.dma_start(out=qkb, in_=qkbd)
            qvb = one.tile([128, C], F32, tag="qvb")
            nc.sync.dma_start(out=qvb, in_=qvbd)
            fw = one.tile([128, NOPT, C, OUTP], F32R, tag="fw")
            nc.gpsimd.dma_start(out=fw, in_=fwd)
            fb = one.tile([BL, OUTP], F32, tag="fb")
            nc.sync.dma_start(out=fb, in_=fbd)
            oht = one.tile([LQ, BL], F32, tag="oht")
            nc.sync.dma_start(out=oht, in_=ohd)
            qct = stream.tile([LQ, BL, H], F32, tag="stream")
            for b in range(BL):
                nc.sync.dma_start(out=qct[:, b, :], in_=qcd[b])

            # ---------- big weights ----------
            # wk casts f32->f32r so it must use SWDGE; it heads the SWDGE queue
            # so the first Kp matmuls start as early as possible.  Plain-f32
            # weights ride HWDGE behind the small loads.
            wk = wbig.tile([128, C, H], F32R, tag="w")
            nc.gpsimd.dma_start(
                out=wk[:, :, ts(0, 128)],
                in_=wKa[:, ts(0, 128)].rearrange("(c p) o -> p c o", p=128),
            )
            wq = wbig.tile([128, C, H], F32, tag="w")
            nc.sync.dma_start(out=wq, in_=wQa.rearrange("(c p) o -> p c o", p=128))
            wqv = wbig.tile([128, C, H], F32R, tag="w")

            # ---------- gather oqc via one-hot matmul ----------
            oqcT = one.tile([128, C, BL], F32, tag="oqcT")
            for c in range(C):
                po = psml.tile([128, BL], F32, tag="sml")
                for b in range(BL):
                    nc.tensor.matmul(
                        po[:, b : b + 1],
                        lhsT=qct[:, b, ts(c, 128)],
                        rhs=oht[:, b : b + 1],
                        start=True,
                        stop=True,
                    )
                nc.vector.tensor_copy(oqcT[:, c, :], po)

            # ---------- Qp^T = aQw @ oqc^T ; article tanh bias ----------
            biasA = one.tile([128, C, BL], F32, tag="biasA")
            for co in range(C):
                pq = psml.tile([128, BL], F32, tag="sml")
                for ci in range(C):
                    nc.tensor.matmul(
                        pq,
                        lhsT=wq[:, ci, ts(co, 128)],
                        rhs=oqcT[:, ci, :],
                        start=(ci == 0),
                        stop=(ci == C - 1),
                    )
                nc.vector.tensor_scalar_add(biasA[:, co, :], pq, qkb[:, co : co + 1])

            # ---------- article branch ----------
            s_sums = one.tile([1, BL, NLT], F32, tag="s_sums")
            uTun = one.tile([128, C, BL], F32, tag="uTun")
            wdk = wbig.tile([128, C, H], F32R, tag="w")
            for b in range(BL):
                upart = ubuf.tile([128, C, NLT], F32, tag="upart")
                for lt in range(NLT):
                    T = stream.tile([128, C, LT], F32R, tag="stream")
                    nc.gpsimd.dma_start(
                        out=T,
                        in_=artT[b, :, ds(lt * LT, LT)].rearrange(
                            "(c p) l -> p c l", p=128
                        ),
                    )
                    if b == 0 and lt == 0:
                        for cw in range(1, C):
                            nc.gpsimd.dma_start(
                                out=wk[:, :, ts(cw, 128)],
                                in_=wKa[:, ts(cw, 128)].rearrange(
                                    "(c p) o -> p c o", p=128
                                ),
                            )
                    lg = prow.tile([1, LT], F32, tag="lg")
                    for co in range(C):
                        kp = pacc.tile([128, LT], F32, tag="acc")
                        for ci in range(C):
                            nc.tensor.matmul(
                                kp,
                                lhsT=wk[:, ci, ts(co, 128)],
                                rhs=T[:, ci, :],
                                start=(ci == 0),
                                stop=(ci == C - 1),
                            )
                        mt = mpool.tile([128, LT], F32R, tag="mt")
                        nc.scalar.activation(
                            mt, kp, AF.Tanh, bias=biasA[:, co, b : b + 1]
                        )
                        nc.tensor.matmul(
                            lg,
                            lhsT=vwa[:, co : co + 1],
                            rhs=mt,
                            start=(co == 0),
                            stop=(co == C - 1),
                        )
                    st = spool.tile([1, LT], F32R, tag="st")
                    nc.scalar.activation(
                        st, lg, AF.Exp, accum_out=s_sums[:, b, lt : lt + 1]
                    )
                    # replicate s~ across partitions: ones^T (x) st via PE
                    prep = pacc.tile([128, LT], F32, tag="acc")
                    nc.tensor.matmul(prep, lhsT=ones, rhs=st, start=True, stop=True)
                    srep = rpool.tile([128, LT], F32, tag="srep")
                    nc.scalar.copy(srep, prep)
                    CH = C // 2
                    for hh in range(2):
                        scr = scratch.tile([128, CH, LT], F32, tag="scr")
                        nc.vector.tensor_mul(
                            scr,
                            T[:, ds(hh * CH, CH)].bitcast(F32),
                            srep.unsqueeze(1).broadcast_to((128, CH, LT)),
                        )
                        nc.vector.tensor_reduce(
                            upart[:, ds(hh * CH, CH), lt : lt + 1],
                            scr,
                            axis=AX.X,
                            op=ALU.add,
                        )
                if b == 0:
                    # emit the wqv/dKw loads after b0's article tiles are
                    # queued: they ride SWDGE behind them and land mid-article,
                    # well before the options phase needs them.
                    nc.gpsimd.dma_start(
                        out=wqv, in_=wQV.rearrange("(c p) o -> p c o", p=128)
                    )
                    nc.gpsimd.dma_start(
                        out=wdk, in_=wKd.rearrange("(c p) o -> p c o", p=128)
                    )
                # sum the NLT partial weighted sums -> unnormalized u^T
                nc.vector.tensor_reduce(
                    uTun[:, :, b : b + 1], upart, axis=AX.X, op=ALU.add
                )

            # normalization factors: 1/sum(exp) per b, replicated to 128 parts
            ssb = one.tile([1, BL], F32, tag="ssb")
            nc.vector.tensor_reduce(ssb, s_sums, axis=AX.X, op=ALU.add)
            psb = psml.tile([128, BL], F32, tag="sml")
            nc.tensor.matmul(
                psb, lhsT=ones.bitcast(F32), rhs=ssb, start=True, stop=True
            )
            rs_rep = one.tile([128, BL], F32, tag="rs_rep")
            nc.vector.reciprocal(rs_rep, psb)

            uT = one.tile([128, C, BL], F32R, tag="uT")
            for b in range(BL):
                nc.vector.tensor_scalar_mul(
                    uT[:, :, b], uTun[:, :, b], rs_rep[:, b : b + 1]
                )

            # ---------- option tanh bias via folded Wqv = d_Qw a_Vw^T ----------
            biasO = one.tile([128, C, BL], F32, tag="biasO")
            for co in range(C):
                pq2 = psml.tile([128, BL], F32, tag="sml")
                for ci in range(C):
                    nc.tensor.matmul(
                        pq2,
                        lhsT=wqv[:, ci, ts(co, 128)],
                        rhs=uT[:, ci, :],
                        start=(ci == 0),
                        stop=(ci == C - 1),
                    )
                nc.vector.tensor_scalar_add(biasO[:, co, :], pq2, qvb[:, co : co + 1])

            # ---------- options branch ----------
            OT = stream.tile([128, C, BL, NOPT, LO], F32R, tag="stream")
            for b in range(BL):
                nc.gpsimd.dma_start(
                    out=OT[:, :, b],
                    in_=optT[b].rearrange("(c p) o l -> p c o l", p=128),
                )
            mdt = stream.tile([128, C, BL, NOPT, LO], F32R, tag="stream")
            HALF = 2 * NOPT * LO  # 320 columns (2 batch items)
            for co in range(C):
                for h in range(2):
                    kpd = pacc.tile([128, HALF], F32, tag="acc")
                    for ci in range(C):
                        nc.tensor.matmul(
                            kpd,
                            lhsT=wdk[:, ci, ts(co, 128)],
                            rhs=OT[:, ci, ds(2 * h, 2)],
                            start=(ci == 0),
                            stop=(ci == C - 1),
                        )
                    for bq in range(2):
                        b = 2 * h + bq
                        nc.scalar.activation(
                            mdt[:, co, b],
                            kpd[:, ds(bq * NOPT * LO, NOPT * LO)],
                            AF.Tanh,
                            bias=biasO[:, co, b : b + 1],
                        )

            s_d = one.tile([1, BO * LO], F32R, tag="s_d")
            for h in range(2):
                lgd = prow.tile([1, HALF], F32, tag="lg")
                for co in range(C):
                    nc.tensor.matmul(
                        lgd,
                        lhsT=vwd[:, co : co + 1],
                        rhs=mdt[:, co, ds(2 * h, 2)],
                        start=(co == 0),
                        stop=(co == C - 1),
                    )
                nc.scalar.activation(s_d[:, ds(h * HALF, HALF)], lgd, AF.Exp)

            sums_d = one.tile([1, BO], F32, tag="sums_d")
            nc.vector.tensor_reduce(
                sums_d,
                s_d.bitcast(F32).rearrange("p (bo l) -> p bo l", l=LO),
                axis=AX.X,
                op=ALU.add,
            )
            rec_d = one.tile([1, BO], F32, tag="rec_d")
            nc.vector.reciprocal(rec_d, sums_d)
            # replicate raw exp scores and 1/sum across partitions
            sdrep = rdpool.tile([128, BO * LO], F32, tag="sdrep")
            for h in range(2):
                prepd = pacc.tile([128, HALF], F32, tag="acc")
                nc.tensor.matmul(
                    prepd,
                    lhsT=ones,
                    rhs=s_d[:, ds(h * HALF, HALF)],
                    start=True,
                    stop=True,
                )
                nc.scalar.copy(sdrep[:, ds(h * HALF, HALF)], prepd)
            prec = psml.tile([128, BO], F32, tag="sml")
            nc.tensor.matmul(
                prec, lhsT=ones.bitcast(F32), rhs=rec_d, start=True, stop=True
            )
            rec_rep = one.tile([128, BO], F32, tag="rec_rep")
            nc.scalar.copy(rec_rep, prec)

            # weighted V-sum, normalize, and final linear - interleaved per c
            u_un = one.tile([128, C, BO], F32, tag="u_un")
            u_dT = one.tile([128, C, BO], F32R, tag="u_dT")
            OTf = OT.bitcast(F32).rearrange("p c b o l -> p c (b o) l")
            sdv = sdrep.rearrange("p (bo l) -> p bo l", l=LO)
            pout = psml.tile([BL, OUTP], F32, tag="sml")
            uv = u_dT.rearrange("p c (b o) -> p c b o", o=NOPT)
            for c in range(C):
                scrd = scratch.tile([128, BO, LO], F32, tag="scr")
                nc.vector.tensor_mul(scrd, OTf[:, c], sdv)
                nc.vector.tensor_reduce(
                    u_un[:, c : c + 1, :].rearrange("p one bo -> p bo one"),
                    scrd,
                    axis=AX.X,
                    op=ALU.add,
                )
                nc.vector.tensor_mul(u_dT[:, c, :], u_un[:, c, :], rec_rep)
                for o in range(NOPT):
                    nc.tensor.matmul(
                        pout,
                        lhsT=uv[:, c, :, o],
                        rhs=fw[:, o, c, :],
                        start=(c == 0 and o == 0),
                        stop=(c == C - 1 and o == NOPT - 1),
                    )
            out_s = one.tile([BL, OUTP], F32, tag="out_s")
            nc.vector.tensor_add(out_s, pout, fb)
            nc.sync.dma_start(out=outd, in_=out_s[:, 0:OUT])

    nc.compile()
    return nc


@functools.lru_cache(maxsize=1)
def get_nc() -> bass.Bass:
    return build_nc()


def make_in_maps(inputs: dict) -> list[dict]:
    art = np.ascontiguousarray(np.asarray(inputs["article_contexts"], np.float32))
    qc = np.ascontiguousarray(np.asarray(inputs["question_contexts"], np.float32))
    opt = np.ascontiguousarray(np.asarray(inputs["options_embeds"], np.float32))
    idx = np.asarray(inputs["answer_indices"]).astype(np.int64)

    def g(name):
        return np.asarray(inputs[name], np.float32)

    aQwT = np.ascontiguousarray(g("a_Qw").T)
    aKwT = np.ascontiguousarray(g("a_Kw").T)
    dKwT = np.ascontiguousarray(g("d_Kw").T)
    # folded: aq -> options query projection
    Wqv = g("d_Qw") @ g("a_Vw")  # [H, H] (a_Vw maps h_in->h_out as aq = u @ a_Vw^T)
    qvwT = np.ascontiguousarray(Wqv.T).astype(np.float32)
    bias_qv = g("d_Qw") @ g("a_Vb") + g("d_Qb") + g("d_Kb")  # [H]
    # folded: per-option final weights
    # feats[b,o,:] = u_d[b,o] @ d_Vw^T + d_Vb ; logits = sum_o feats[b,o] @ f_w[:,o]^T + f_b
    # => logits = sum_o u_d[b,o] @ (d_Vw^T @ f_w[:,o]^T) + (f_b + sum_o f_w[:,o] @ d_Vb)
    f_w = g("f_w")  # [OUT, 5H], flattened o-major
    dVwT = g("d_Vw").T  # [H_in, H_out]
    Ff = np.stack(
        [dVwT @ f_w[:, o * H : (o + 1) * H].T for o in range(NOPT)], axis=0
    )  # [o, H_in, OUT]
    fb_new = g("f_b") + sum(
        f_w[:, o * H : (o + 1) * H] @ g("d_Vb") for o in range(NOPT)
    )  # [OUT]
    fwT = np.zeros((128, NOPT, C, 8), np.float32)
    fwT[:, :, :, :OUT] = Ff.reshape(NOPT, C, 128, OUT).transpose(2, 0, 1, 3)

    def colvec(v):  # [H] -> [128, C] chunk-major
        return np.ascontiguousarray(np.asarray(v, np.float32).reshape(C, 128).T)

    vwaT = colvec(g("a_vw").reshape(H))
    vwdT = colvec(g("d_vw").reshape(H))
    qkbT = colvec(g("a_Qb") + g("a_Kb"))
    qvbT = colvec(bias_qv)

    artT = np.ascontiguousarray(art.transpose(0, 2, 1))  # [B, H, LA]
    optT = np.ascontiguousarray(opt.transpose(0, 3, 1, 2))  # [B, H, 5, LO]
    onehot = np.zeros((B, LQ), np.float32)
    onehot[np.arange(B), idx] = 1.0

    shared = dict(
        aQwT=aQwT, aKwT=aKwT, qvwT=qvwT, dKwT=dKwT,
        vwaT=vwaT, vwdT=vwdT, qkbT=qkbT, qvbT=qvbT,
        fwT=fwT,
        fb=np.ascontiguousarray(
            np.tile(
                np.pad(fb_new.astype(np.float32), (0, 3)).reshape(1, 8), (BL, 1)
            )
        ),
        ones1=np.ones((1, 128), np.float32),
    )
    in_maps = []
    for r in range(NCORES):
        s = slice(r * BL, (r + 1) * BL)
        m = dict(shared)
        m["artT"] = artT[s]
        m["optT"] = optT[s]
        m["qc"] = qc[s]
        m["oh"] = np.ascontiguousarray(onehot[s].T)
        in_maps.append(m)
    return in_maps


def run(inputs: dict, trace: bool = False, tmpdir=None):
    from concourse.bass_utils import run_bass_kernel_spmd

    nc = get_nc()
    in_maps = make_in_maps(inputs)
    res = run_bass_kernel_spmd(
        nc, in_maps, core_ids=list(range(NCORES)), trace=trace, tmpdir=tmpdir
    )
    out = np.concatenate([res.results[r]["out"] for r in range(NCORES)], axis=0)
    return out, res


def kernel(**inputs) -> np.ndarray:
    out, _ = run(inputs, trace=False)
    return out



# revision 3
# speedup vs baseline: 1.6674x; 1.6674x over previous
"""Trainium2 Bass kernel for nn_BaselineOut (article/option additive-attention MRC head).

Contract: kernel(**inputs) takes FULL unsharded inputs (numpy), returns FULL
[32, 5] float32 logits.  Internally: data-parallel over batch across 8 cores
(4 batch items per core), all params replicated.

Math notes (vs reference):
  - oqc gather is done as a one-hot matmul on device.
  - V-projection is pulled out of the attention sum by linearity:
        sum_l softmax_l * (V @ Vw^T + Vb) = (sum_l softmax_l * V) @ Vw^T + Vb
  - Consecutive linear maps with no nonlinearity between are constant-folded
    on host (weight-weight products):
      * aq -> Qp_d: one matmul with Wqv = d_Qw @ a_Vw^T and a folded bias.
      * feats -> logits: per-option folded weights Ff_o = d_Vw^T @ f_w[:,o]^T.
  - softmax logit bias (vb) is dropped: softmax is shift-invariant.
  - exp is computed without max-subtraction: |logit| <= ||vw||_1 ~ 36, well
    inside fp32 exp range.

Precision (validated vs reference on CPU, rel err ~5e-3 vs 2e-2 budget):
  - article K-projection (the 17 GFLOP/core matmul) runs in fp8-e4m3 with
    DoubleRow perf mode (2 fp8 weights per PE cell, ~2x matmul throughput);
    the article values feeding the weighted V-sum are the same fp8 copy.
  - options branch + all small matmuls run in bf16 (fp8 there fails: softmax
    over only 32 positions doesn't average the noise away).
  - accumulation is fp32 in PSUM everywhere; softmax sums/normalization fp32.

Scheduling:
  - options K-projection (kpd) has no dependency on the article branch; its
    co-blocks are interleaved into the article tile loop so the tail chain is
    short (bias -> tanh -> scores -> softmax -> V-sum -> folded final linear).
  - score replication across partitions uses gpsimd partition_broadcast, off
    the PE/scalar critical path.
  - biasA matmuls are interleaved into tile 0's co-loop so the first article
    tile doesn't wait for the full a_Qw load.
"""

import functools
import sys

import numpy as np

sys.path.insert(0, "/opt/trn_rl_repo")

import ml_dtypes  # noqa: E402

import concourse.bass as bass  # noqa: E402
from concourse import bacc  # noqa: E402
import concourse.tile as tile  # noqa: E402
from concourse import mybir  # noqa: E402
from concourse.bass import ds, ts  # noqa: E402

B, LA, LQ, LO, H, OUT = 32, 2048, 64, 32, 1024, 5
NCORES = 8
BL = B // NCORES  # 4 batch items per core
NOPT = 5
F32 = mybir.dt.float32
BF16 = mybir.dt.bfloat16
FP8 = mybir.dt.float8e4
LH = 1024  # article l-tile (half of one batch item's article)
HT = LA // LH  # 2 halves per batch item
C = H // 128  # 8 h-chunks
CP = C // 2  # 4 ci pairs for DoubleRow
BO = BL * NOPT  # 20 (b, option) pairs per core
OL = BO * LO  # 640 flattened option columns
AF = mybir.ActivationFunctionType
ALU = mybir.AluOpType
AX = mybir.AxisListType
OUTP = 8  # final-linear out dim padded
DR = mybir.MatmulPerfMode.DoubleRow


def build_nc() -> bass.Bass:
    nc = bacc.Bacc("TRN2", target_bir_lowering=False, debug=False)

    # ---- DRAM I/O (per-core shard; names are the in_map keys) ----
    art8 = nc.dram_tensor("art8", [BL, H, LA], FP8, kind="ExternalInput").ap()
    optT = nc.dram_tensor("optT", [BL, H, NOPT, LO], BF16, kind="ExternalInput").ap()
    qcd = nc.dram_tensor("qc", [BL, LQ, H], BF16, kind="ExternalInput").ap()
    ohd = nc.dram_tensor("oh", [LQ, BL], BF16, kind="ExternalInput").ap()
    wQa = nc.dram_tensor("aQwT", [H, H], BF16, kind="ExternalInput").ap()
    wKa = nc.dram_tensor("aKwT", [H, H], FP8, kind="ExternalInput").ap()
    wQV = nc.dram_tensor("qvwT", [H, H], BF16, kind="ExternalInput").ap()
    wKd = nc.dram_tensor("dKwT", [H, H], BF16, kind="ExternalInput").ap()
    vwad = nc.dram_tensor("vwaT", [128, C], BF16, kind="ExternalInput").ap()
    vwdd = nc.dram_tensor("vwdT", [128, C], BF16, kind="ExternalInput").ap()
    qkbd = nc.dram_tensor("qkbT", [128, C], F32, kind="ExternalInput").ap()
    qvbd = nc.dram_tensor("qvbT", [128, C], F32, kind="ExternalInput").ap()
    fwd = nc.dram_tensor("fwT", [128, NOPT, C, OUTP], BF16, kind="ExternalInput").ap()
    fbd = nc.dram_tensor("fb", [BL, OUTP], F32, kind="ExternalInput").ap()
    outd = nc.dram_tensor("out", [BL, OUT], F32, kind="ExternalOutput").ap()

    with (
        tile.TileContext(nc) as tc,
        nc.allow_low_precision(reason="fp8/bf16 operands; PE accumulates fp32"),
    ):
        with (
            tc.tile_pool(name="one", bufs=1) as one,
            tc.tile_pool(name="stream", bufs=3) as stream,
            tc.tile_pool(name="mtp", bufs=3) as mtp,
            tc.tile_pool(name="stp", bufs=2) as stp,
            tc.tile_pool(name="srp", bufs=2) as srp,
            tc.tile_pool(name="scrp", bufs=1) as scrp,
            tc.tile_pool(name="pacc", bufs=2, space="PSUM") as pacc,
            tc.tile_pool(name="prow", bufs=1, space="PSUM") as prow,
            tc.tile_pool(name="pkpd", bufs=1, space="PSUM") as pkpd,
            tc.tile_pool(name="psml", bufs=1, space="PSUM") as psml,
        ):
            # ---------- sync (HWDGE) queue: small consts, qc, weights ----------
            vwa = one.tile([128, C], BF16, tag="vwa")
            nc.sync.dma_start(out=vwa, in_=vwad)
            vwd = one.tile([128, C], BF16, tag="vwd")
            nc.sync.dma_start(out=vwd, in_=vwdd)
            qkb = one.tile([128, C], F32, tag="qkb")
            nc.sync.dma_start(out=qkb, in_=qkbd)
            qvb = one.tile([128, C], F32, tag="qvb")
            nc.sync.dma_start(out=qvb, in_=qvbd)
            fw = one.tile([128, NOPT, C, OUTP], BF16, tag="fw")
            nc.sync.dma_start(out=fw, in_=fwd)
            fb = one.tile([BL, OUTP], F32, tag="fb")
            nc.sync.dma_start(out=fb, in_=fbd)
            oht = one.tile([LQ, BL], BF16, tag="oht")
            nc.sync.dma_start(out=oht, in_=ohd)
            qct = one.tile([LQ, BL, H], BF16, tag="qct")
            for b in range(BL):
                nc.sync.dma_start(out=qct[:, b, :], in_=qcd[b])
            # a_Qw in per-co column chunks so biasA(co) can start early
            wq = one.tile([128, C, H], BF16, tag="wq")
            for cw in range(C):
                nc.sync.dma_start(
                    out=wq[:, :, ts(cw, 128)],
                    in_=wQa[:, ts(cw, 128)].rearrange("(c p) o -> p c o", p=128),
                )
            # then the options-phase weights (needed from tile ~1 onwards)
            wdk = one.tile([128, C, H], BF16, tag="wdk")
            nc.sync.dma_start(out=wdk, in_=wKd.rearrange("(c p) o -> p c o", p=128))
            OT = one.tile([128, C, BL, NOPT, LO], BF16, tag="OT")
            for b in range(BL):
                nc.sync.dma_start(
                    out=OT[:, :, b],
                    in_=optT[b].rearrange("(c p) o l -> p c o l", p=128),
                )
            wqv = one.tile([128, C, H], BF16, tag="wqv")
            nc.sync.dma_start(out=wqv, in_=wQV.rearrange("(c p) o -> p c o", p=128))

            # ---------- gpsimd (SWDGE) queue: fp8 article weights + stream ----
            wk8 = one.tile([128, C, H], FP8, tag="wk8")
            nc.gpsimd.dma_start(
                out=wk8[:, :, ts(0, 128)],
                in_=wKa[:, ts(0, 128)].rearrange("(c p) o -> p c o", p=128),
            )

            # ---------- persistent accumulators / small tensors ----------
            oqcT = one.tile([128, C, BL], BF16, tag="oqcT")
            biasA = one.tile([128, C, BL], F32, tag="biasA")
            biasO = one.tile([128, C, BL], F32, tag="biasO")
            s_sums = one.tile([1, BL, HT], F32, tag="s_sums")
            upart = one.tile([128, C, HT], F32, tag="upart")
            uTun = one.tile([128, C, BL], F32, tag="uTun")
            uT = one.tile([128, C, BL], BF16, tag="uT")
            mdt_pre = one.tile([128, C, OL], BF16, tag="mdt_pre")

            # ---------- gather oqc via one-hot matmul ----------
            for c in range(C):
                po = psml.tile([128, BL], F32, tag="sml")
                for b in range(BL):
                    nc.tensor.matmul(
                        po[:, b : b + 1],
                        lhsT=qct[:, b, ts(c, 128)],
                        rhs=oht[:, b : b + 1],
                        start=True,
                        stop=True,
                    )
                nc.vector.tensor_copy(oqcT[:, c, :], po)

            # ---------- article branch ----------
            def bias_a_block(co):
                # biasA[:, co, :] = a_Qw @ oqc^T + (a_Qb + a_Kb), chunk co
                pq = psml.tile([128, BL], F32, tag="sml")
                for ci in range(C):
                    nc.tensor.matmul(
                        pq,
                        lhsT=wq[:, ci, ts(co, 128)],
                        rhs=oqcT[:, ci, :],
                        start=(ci == 0),
                        stop=(ci == C - 1),
                    )
                nc.vector.tensor_scalar_add(biasA[:, co, :], pq, qkb[:, co : co + 1])

            def kpd_block(co):
                # options K-projection chunk co -> mdt_pre (pre-tanh, no bias)
                OTf = OT.rearrange("p c b o l -> p c (b o l)")
                for hh in range(2):
                    kpd = pkpd.tile([128, OL // 2], F32, tag="kpd")
                    for ci in range(C):
                        nc.tensor.matmul(
                            kpd,
                            lhsT=wdk[:, ci, ts(co, 128)],
                            rhs=OTf[:, ci, ds(hh * (OL // 2), OL // 2)],
                            start=(ci == 0),
                            stop=(ci == C - 1),
                        )
                    nc.vector.tensor_copy(
                        mdt_pre[:, co, ds(hh * (OL // 2), OL // 2)], kpd
                    )

            tile_idx = 0
            for b in range(BL):
                for h in range(HT):
                    T8 = stream.tile([128, C, LH], FP8, tag="stream")
                    nc.gpsimd.dma_start(
                        out=T8,
                        in_=art8[b, :, ds(h * LH, LH)].rearrange(
                            "(c p) l -> p c l", p=128
                        ),
                    )
                    if tile_idx == 0:
                        # rest of the fp8 K-weights, behind tile 0's article
                        for cw in range(1, C):
                            nc.gpsimd.dma_start(
                                out=wk8[:, :, ts(cw, 128)],
                                in_=wKa[:, ts(cw, 128)].rearrange(
                                    "(c p) o -> p c o", p=128
                                ),
                            )
                    lg = prow.tile([1, LH], F32, tag="lg")
                    for co in range(C):
                        if tile_idx == 0:
                            bias_a_block(co)
                        kp = pacc.tile([128, LH], F32, tag="acc")
                        for lt in range(2):
                            for cp in range(CP):
                                nc.tensor.matmul(
                                    kp[:, ds(lt * 512, 512)],
                                    lhsT=wk8[:, ds(2 * cp, 2), ts(co, 128)],
                                    rhs=T8[:, ds(2 * cp, 2), ds(lt * 512, 512)],
                                    start=(cp == 0),
                                    stop=(cp == CP - 1),
                                    perf_mode=DR,
                                )
                        mt = mtp.tile([128, LH], BF16, tag="mt")
                        nc.scalar.activation(
                            mt, kp, AF.Tanh, bias=biasA[:, co, b : b + 1]
                        )
                        for lt in range(2):
                            nc.tensor.matmul(
                                lg[:, ds(lt * 512, 512)],
                                lhsT=vwa[:, co : co + 1],
                                rhs=mt[:, ds(lt * 512, 512)],
                                start=(co == 0),
                                stop=(co == C - 1),
                            )
                    st = stp.tile([1, LH], BF16, tag="st")
                    nc.scalar.activation(
                        st, lg, AF.Exp, accum_out=s_sums[:, b, h : h + 1]
                    )
                    srep = srp.tile([128, LH], BF16, tag="srep")
                    nc.gpsimd.partition_broadcast(srep, st)
                    scr = scrp.tile([128, C, LH], BF16, tag="scr")
                    nc.vector.tensor_mul(
                        scr, T8, srep.unsqueeze(1).broadcast_to((128, C, LH))
                    )
                    nc.vector.tensor_reduce(
                        upart[:, :, h : h + 1], scr, axis=AX.X, op=ALU.add
                    )
                    if tile_idx >= 1:
                        kpd_block(tile_idx - 1)
                    tile_idx += 1
                nc.vector.tensor_reduce(
                    uTun[:, :, b : b + 1], upart, axis=AX.X, op=ALU.add
                )
            kpd_block(C - 1)

            # ---------- article normalization ----------
            ssb = one.tile([1, BL, 1], F32, tag="ssb")
            nc.vector.tensor_reduce(ssb, s_sums, axis=AX.X, op=ALU.add)
            rsb = one.tile([1, BL], F32, tag="rsb")
            nc.vector.reciprocal(rsb, ssb.rearrange("p b one -> p (b one)"))
            rs_rep = one.tile([128, BL], F32, tag="rs_rep")
            nc.gpsimd.partition_broadcast(rs_rep, rsb)
            nc.vector.tensor_mul(
                uT, uTun, rs_rep.unsqueeze(1).broadcast_to((128, C, BL))
            )

            # ---------- option tanh bias via folded Wqv = d_Qw a_Vw^T ----------
            for co in range(C):
                pq2 = psml.tile([128, BL], F32, tag="sml")
                for ci in range(C):
                    nc.tensor.matmul(
                        pq2,
                        lhsT=wqv[:, ci, ts(co, 128)],
                        rhs=uT[:, ci, :],
                        start=(ci == 0),
                        stop=(ci == C - 1),
                    )
                nc.vector.tensor_scalar_add(biasO[:, co, :], pq2, qvb[:, co : co + 1])

            # ---------- options branch tail ----------
            # mdt = tanh(mdt_pre + biasO), pipelined in two b-halves
            mdt = scrp.tile([128, C, LH], BF16, tag="scr")
            HB = OL // 2  # 320 columns = 2 batch items
            lgd = prow.tile([1, 2, 512], F32, tag="lg")  # bank-aligned halves
            for bh in range(2):
                sl = ds(bh * HB, HB)
                nc.vector.tensor_add(
                    mdt[:, :, sl].rearrange("p c (b ol) -> p c b ol", b=2),
                    mdt_pre[:, :, sl].rearrange("p c (b ol) -> p c b ol", b=2),
                    biasO[:, :, ds(bh * 2, 2)]
                    .unsqueeze(-1)
                    .broadcast_to((128, C, 2, NOPT * LO)),
                )
                nc.scalar.activation(mdt[:, :, sl], mdt[:, :, sl], AF.Tanh)
                for co in range(C):
                    nc.tensor.matmul(
                        lgd[:, bh, 0:HB],
                        lhsT=vwd[:, co : co + 1],
                        rhs=mdt[:, co, sl],
                        start=(co == 0),
                        stop=(co == C - 1),
                    )
            s_d = one.tile([1, OL], BF16, tag="s_d")
            nc.scalar.activation(
                s_d.rearrange("p (bh x) -> p bh x", bh=2), lgd[:, :, 0:HB], AF.Exp
            )
            sums_d = one.tile([1, BO, 1], F32, tag="sums_d")
            nc.vector.tensor_reduce(
                sums_d,
                s_d.rearrange("p (bo l) -> p bo l", l=LO),
                axis=AX.X,
                op=ALU.add,
            )
            rec_d = one.tile([1, BO], F32, tag="rec_d")
            nc.vector.reciprocal(rec_d, sums_d.rearrange("p bo one -> p (bo one)"))
            sdn = one.tile([1, OL], BF16, tag="sdn")
            nc.vector.tensor_mul(
                sdn.rearrange("p (bo l) -> p bo l", l=LO),
                s_d.rearrange("p (bo l) -> p bo l", l=LO),
                rec_d.unsqueeze(-1).broadcast_to((1, BO, LO)),
            )
            sdrep = srp.tile([128, LH], BF16, tag="srep")
            nc.gpsimd.partition_broadcast(sdrep[:, 0:OL], sdn)

            # normalized weighted V-sum over options + folded final linear
            scrd = scrp.tile([128, C, LH], BF16, tag="scr")
            OTf = OT.rearrange("p c b o l -> p c (b o l)")
            nc.vector.tensor_mul(
                scrd[:, :, 0:OL],
                OTf,
                sdrep[:, 0:OL].unsqueeze(1).broadcast_to((128, C, OL)),
            )
            u_d = one.tile([128, C, BO, 1], F32, tag="u_d")
            nc.vector.tensor_reduce(
                u_d,
                scrd[:, :, 0:OL].rearrange("p c (bo l) -> p c bo l", l=LO),
                axis=AX.X,
                op=ALU.add,
            )
            u16 = one.tile([128, C, BL, NOPT], BF16, tag="u16")
            nc.vector.tensor_copy(
                u16.rearrange("p c b o -> p c (b o)"),
                u_d.rearrange("p c bo one -> p c (bo one)"),
            )
            pout = psml.tile([BL, OUTP], F32, tag="sml")
            for c in range(C):
                for o in range(NOPT):
                    nc.tensor.matmul(
                        pout,
                        lhsT=u16[:, c, :, o],
                        rhs=fw[:, o, c, :],
                        start=(c == 0 and o == 0),
                        stop=(c == C - 1 and o == NOPT - 1),
                    )
            out_s = one.tile([BL, OUTP], F32, tag="out_s")
            nc.vector.tensor_add(out_s, pout, fb)
            nc.sync.dma_start(out=outd, in_=out_s[:, 0:OUT])

    nc.compile()
    return nc


@functools.lru_cache(maxsize=1)
def get_nc() -> bass.Bass:
    return build_nc()


def make_in_maps(inputs: dict) -> list[dict]:
    bf16 = ml_dtypes.bfloat16
    fp8 = ml_dtypes.float8_e4m3
    art = np.ascontiguousarray(np.asarray(inputs["article_contexts"], np.float32))
    qc = np.asarray(inputs["question_contexts"], np.float32)
    opt = np.ascontiguousarray(np.asarray(inputs["options_embeds"], np.float32))
    idx = np.asarray(inputs["answer_indices"]).astype(np.int64)

    def g(name):
        return np.asarray(inputs[name], np.float32)

    aQwT = np.ascontiguousarray(g("a_Qw").T).astype(bf16)
    aKwT = np.ascontiguousarray(g("a_Kw").T).astype(fp8)
    dKwT = np.ascontiguousarray(g("d_Kw").T).astype(bf16)
    # folded: aq -> options query projection
    Wqv = g("d_Qw") @ g("a_Vw")  # [H, H]
    qvwT = np.ascontiguousarray(Wqv.T).astype(bf16)
    bias_qv = g("d_Qw") @ g("a_Vb") + g("d_Qb") + g("d_Kb")  # [H]
    # folded: per-option final weights
    f_w = g("f_w")  # [OUT, 5H]
    dVwT = g("d_Vw").T
    Ff = np.stack(
        [dVwT @ f_w[:, o * H : (o + 1) * H].T for o in range(NOPT)], axis=0
    )  # [o, H_in, OUT]
    fb_new = g("f_b") + sum(
        f_w[:, o * H : (o + 1) * H] @ g("d_Vb") for o in range(NOPT)
    )  # [OUT]
    fwT = np.zeros((128, NOPT, C, OUTP), np.float32)
    fwT[:, :, :, :OUT] = Ff.reshape(NOPT, C, 128, OUT).transpose(2, 0, 1, 3)

    def colvec(v, dt):  # [H] -> [128, C] chunk-major
        return np.ascontiguousarray(
            np.asarray(v, np.float32).reshape(C, 128).T
        ).astype(dt)

    vwaT = colvec(g("a_vw").reshape(H), bf16)
    vwdT = colvec(g("d_vw").reshape(H), bf16)
    qkbT = colvec(g("a_Qb") + g("a_Kb"), np.float32)
    qvbT = colvec(bias_qv, np.float32)

    artT = np.ascontiguousarray(art.transpose(0, 2, 1)).astype(fp8)  # [B, H, LA]
    optT = np.ascontiguousarray(opt.transpose(0, 3, 1, 2)).astype(bf16)
    onehot = np.zeros((B, LQ), np.float32)
    onehot[np.arange(B), idx] = 1.0

    shared = dict(
        aQwT=aQwT, aKwT=aKwT, qvwT=qvwT, dKwT=dKwT,
        vwaT=vwaT, vwdT=vwdT, qkbT=qkbT, qvbT=qvbT,
        fwT=fwT.astype(bf16),
        fb=np.ascontiguousarray(
            np.tile(np.pad(fb_new.astype(np.float32), (0, 3)).reshape(1, 8), (BL, 1))
        ),
    )
    in_maps = []
    for r in range(NCORES):
        s = slice(r * BL, (r + 1) * BL)
        m = dict(shared)
        m["art8"] = artT[s]
        m["optT"] = optT[s]
        m["qc"] = qc[s].astype(bf16)
        m["oh"] = np.ascontiguousarray(onehot[s].T).astype(bf16)
        in_maps.append(m)
    return in_maps


def run(inputs: dict, trace: bool = False, tmpdir=None):
    from concourse.bass_utils import run_bass_kernel_spmd

    nc = get_nc()
    in_maps = make_in_maps(inputs)
    res = run_bass_kernel_spmd(
        nc, in_maps, core_ids=list(range(NCORES)), trace=trace, tmpdir=tmpdir
    )
    out = np.concatenate([res.results[r]["out"] for r in range(NCORES)], axis=0)
    return out, res


def kernel(**inputs) -> np.ndarray:
    out, _ = run(inputs, trace=False)
    return out


# revision 20
# speedup vs baseline: 1.7680x; 1.0604x over previous
"""Trainium2 Bass kernel for nn_BaselineOut (article/option additive-attention MRC head).

Contract: kernel(**inputs) takes FULL unsharded inputs (numpy), returns FULL
[32, 5] float32 logits.  Internally: data-parallel over batch across 8 cores
(4 batch items per core), all params replicated.

Math notes (vs reference):
  - oqc gather is done as a one-hot matmul on device.
  - V-projection is pulled out of the attention sum by linearity:
        sum_l softmax_l * (V @ Vw^T + Vb) = (sum_l softmax_l * V) @ Vw^T + Vb
  - Consecutive linear maps with no nonlinearity between are constant-folded
    on host (weight-weight products):
      * aq -> Qp_d: one matmul with Wqv = d_Qw @ a_Vw^T and a folded bias.
      * feats -> logits: per-option folded weights Ff_o = d_Vw^T @ f_w[:,o]^T.
  - softmax logit bias (vb) is dropped: softmax is shift-invariant.
  - exp is computed without max-subtraction: |logit| <= ||vw||_1 ~ 36, well
    inside fp32 exp range.

Precision (validated vs reference on CPU, rel err ~5e-3 vs 2e-2 budget):
  - article K-projection (the 17 GFLOP/core matmul) runs in fp8-e4m3 with
    DoubleRow perf mode (2 fp8 weights per PE cell, ~2x matmul throughput);
    the article values feeding the weighted V-sum are the same fp8 copy.
  - options branch + all small matmuls run in bf16 (fp8 there fails: softmax
    over only 32 positions doesn't average the noise away).
  - accumulation is fp32 in PSUM everywhere; softmax sums/normalization fp32.

Scheduling:
  - options K-projection (kpd) has no dependency on the article branch; its
    co-blocks are interleaved into the article tile loop so the tail chain is
    short (bias -> tanh -> scores -> softmax -> V-sum -> folded final linear).
  - score replication across partitions uses gpsimd partition_broadcast, off
    the PE/scalar critical path.
  - biasA matmuls are interleaved into tile 0's co-loop so the first article
    tile doesn't wait for the full a_Qw load.
"""

import functools
import sys

import numpy as np

sys.path.insert(0, "/opt/trn_rl_repo")

import ml_dtypes  # noqa: E402

import concourse.bass as bass  # noqa: E402
from concourse import bacc  # noqa: E402
import concourse.tile as tile  # noqa: E402
from concourse import mybir  # noqa: E402
from concourse.bass import ds, ts  # noqa: E402

B, LA, LQ, LO, H, OUT = 32, 2048, 64, 32, 1024, 5
NCORES = 8
BL = B // NCORES  # 4 batch items per core
NOPT = 5
F32 = mybir.dt.float32
BF16 = mybir.dt.bfloat16
FP8 = mybir.dt.float8e4
LH = 1024  # article l-tile (half of one batch item's article)
HT = LA // LH  # 2 halves per batch item
C = H // 128  # 8 h-chunks
CP = C // 2  # 4 ci pairs for DoubleRow
BO = BL * NOPT  # 20 (b, option) pairs per core
OL = BO * LO  # 640 flattened option columns
AF = mybir.ActivationFunctionType
ALU = mybir.AluOpType
AX = mybir.AxisListType
OUTP = 8  # final-linear out dim padded
DR = mybir.MatmulPerfMode.DoubleRow


def build_nc() -> bass.Bass:
    nc = bacc.Bacc("TRN2", target_bir_lowering=False, debug=False)

    # ---- DRAM I/O (per-core shard; names are the in_map keys) ----
    art8 = nc.dram_tensor("art8", [BL, H, LA], FP8, kind="ExternalInput").ap()
    art16 = nc.dram_tensor("art16", [BL, LA, H], BF16, kind="ExternalInput").ap()
    optT = nc.dram_tensor("optT", [BL, H, NOPT, LO], BF16, kind="ExternalInput").ap()
    qcd = nc.dram_tensor("qc", [BL, LQ, H], BF16, kind="ExternalInput").ap()
    ohd = nc.dram_tensor("oh", [LQ, BL], BF16, kind="ExternalInput").ap()
    wQa = nc.dram_tensor("aQwT", [H, H], BF16, kind="ExternalInput").ap()
    wKa = nc.dram_tensor("aKwT", [H, H], FP8, kind="ExternalInput").ap()
    wQV = nc.dram_tensor("qvwT", [H, H], BF16, kind="ExternalInput").ap()
    wKd = nc.dram_tensor("dKwT", [H, H], BF16, kind="ExternalInput").ap()
    vwad = nc.dram_tensor("vwaT", [128, C], BF16, kind="ExternalInput").ap()
    vwdd = nc.dram_tensor("vwdT", [128, C], BF16, kind="ExternalInput").ap()
    qkbd = nc.dram_tensor("qkbT", [128, C], F32, kind="ExternalInput").ap()
    qvbd = nc.dram_tensor("qvbT", [128, C], F32, kind="ExternalInput").ap()
    fwd = nc.dram_tensor("fwT", [128, NOPT, C, OUTP], BF16, kind="ExternalInput").ap()
    fbd = nc.dram_tensor("fb", [BL, OUTP], F32, kind="ExternalInput").ap()
    outd = nc.dram_tensor("out", [BL, OUT], F32, kind="ExternalOutput").ap()

    with (
        tile.TileContext(nc) as tc,
        nc.allow_low_precision(reason="fp8/bf16 operands; PE accumulates fp32"),
    ):
        with (
            tc.tile_pool(name="one", bufs=1) as one,
            tc.tile_pool(name="stream", bufs=3) as stream,
            tc.tile_pool(name="nstream", bufs=2) as nstream,
            tc.tile_pool(name="mtp", bufs=2) as mtp,
            tc.tile_pool(name="scp", bufs=2) as scp,
            tc.tile_pool(name="srp", bufs=2) as srp,
            tc.tile_pool(name="scrp", bufs=1) as scrp,
            tc.tile_pool(name="pacc", bufs=2, space="PSUM") as pacc,
            tc.tile_pool(name="plgc", bufs=1, space="PSUM") as plgc,
            tc.tile_pool(name="pub", bufs=1, space="PSUM") as pub,
            tc.tile_pool(name="pkpd", bufs=1, space="PSUM") as pkpd,
            tc.tile_pool(name="psml", bufs=1, space="PSUM") as psml,
        ):
            # ---------- sync (HWDGE) queue: qc, weights, smalls, then An ------
            oht = one.tile([LQ, BL], BF16, tag="oht")
            nc.sync.dma_start(out=oht, in_=ohd)
            qct = one.tile([LQ, BL, H], BF16, tag="qct")
            for b in range(BL):
                nc.sync.dma_start(out=qct[:, b, :], in_=qcd[b])
            # a_Qw in per-co column chunks so biasA(co) can start early
            wq = one.tile([128, C, H], BF16, tag="wq")
            for cw in range(C):
                nc.sync.dma_start(
                    out=wq[:, :, ts(cw, 128)],
                    in_=wQa[:, ts(cw, 128)].rearrange("(c p) o -> p c o", p=128),
                )
            vwa = one.tile([128, C], BF16, tag="vwa")
            nc.sync.dma_start(out=vwa, in_=vwad)
            vwd = one.tile([128, C], BF16, tag="vwd")
            nc.sync.dma_start(out=vwd, in_=vwdd)
            qkb = one.tile([128, C], F32, tag="qkb")
            nc.sync.dma_start(out=qkb, in_=qkbd)
            qvb = one.tile([128, C], F32, tag="qvb")
            nc.sync.dma_start(out=qvb, in_=qvbd)
            fw = one.tile([128, NOPT, C, OUTP], BF16, tag="fw")
            nc.sync.dma_start(out=fw, in_=fwd)
            fb = one.tile([BL, OUTP], F32, tag="fb")
            nc.sync.dma_start(out=fb, in_=fbd)

            # ---------- gpsimd (SWDGE) queue: fp8 article weights + stream ----
            wk8 = one.tile([128, C, H], FP8, tag="wk8")
            nc.gpsimd.dma_start(
                out=wk8[:, :, ts(0, 128)],
                in_=wKa[:, ts(0, 128)].rearrange("(c p) o -> p c o", p=128),
            )
            # options weights declared here, loaded inside the loop (tile 1/2)
            wdk = one.tile([128, C, H], BF16, tag="wdk")
            OT = one.tile([128, C, BL, NOPT, LO], BF16, tag="OT")
            wqv = one.tile([128, C, H], BF16, tag="wqv")

            # ---------- persistent accumulators / small tensors ----------
            oqcT = one.tile([128, C, BL], BF16, tag="oqcT")
            biasA = one.tile([128, C, BL], F32, tag="biasA")
            biasO = one.tile([128, C, BL], F32, tag="biasO")
            uTun = one.tile([128, C, BL], F32, tag="uTun")
            uT = one.tile([128, C, BL], BF16, tag="uT")
            mdt_pre = one.tile([128, C, OL], BF16, tag="mdt_pre")
            ones128 = one.tile([128, 1], BF16, tag="ones128")
            nc.vector.memset(ones128, 1.0)
            # u accumulator: per-half-tile columns h*C+co (all groups are
            # closed within one tile: HW start_tensor_calc resets has_written
            # flags bank-wide, so open groups must never interleave with
            # other groups' starts in the same bank); softmax sums in
            # columns 2C+h (partition 0)
            pu = pub.tile([128, 2 * C + HT], F32, tag="pu")
            ssums = one.tile([1, BL], F32, tag="ssums")

            # ---------- gather oqc via one-hot matmul ----------
            for c in range(C):
                po = psml.tile([128, BL], F32, tag="sml")
                for b in range(BL):
                    nc.tensor.matmul(
                        po[:, b : b + 1],
                        lhsT=qct[:, b, ts(c, 128)],
                        rhs=oht[:, b : b + 1],
                        start=True,
                        stop=True,
                    )
                nc.vector.tensor_copy(oqcT[:, c, :], po)

            # ---------- article branch ----------
            def bias_a_block(co):
                # biasA[:, co, :] = a_Qw @ oqc^T + (a_Qb + a_Kb), chunk co
                pq = psml.tile([128, BL], F32, tag="sml")
                for ci in range(C):
                    nc.tensor.matmul(
                        pq,
                        lhsT=wq[:, ci, ts(co, 128)],
                        rhs=oqcT[:, ci, :],
                        start=(ci == 0),
                        stop=(ci == C - 1),
                    )
                nc.vector.tensor_scalar_add(biasA[:, co, :], pq, qkb[:, co : co + 1])

            def kpd_block(co):
                # options K-projection chunk co -> mdt_pre (pre-tanh, no bias)
                OTf = OT.rearrange("p c b o l -> p c (b o l)")
                for hh in range(2):
                    kpd = pkpd.tile([128, OL // 2], F32, tag="kpd")
                    for ci in range(C):
                        nc.tensor.matmul(
                            kpd,
                            lhsT=wdk[:, ci, ts(co, 128)],
                            rhs=OTf[:, ci, ds(hh * (OL // 2), OL // 2)],
                            start=(ci == 0),
                            stop=(ci == C - 1),
                        )
                    nc.vector.tensor_copy(
                        mdt_pre[:, co, ds(hh * (OL // 2), OL // 2)], kpd
                    )

            NLC = LH // 128  # 8 l-chunks of 128 per tile
            tile_idx = 0
            for b in range(BL):
                for h in range(HT):
                    T8 = stream.tile([128, C, LH], FP8, tag="stream")
                    nc.gpsimd.dma_start(
                        out=T8,
                        in_=art8[b, :, ds(h * LH, LH)].rearrange(
                            "(c p) l -> p c l", p=128
                        ),
                    )
                    if tile_idx == 0:
                        # rest of the fp8 K-weights, behind tile 0's article
                        for cw in range(1, C):
                            nc.gpsimd.dma_start(
                                out=wk8[:, :, ts(cw, 128)],
                                in_=wKa[:, ts(cw, 128)].rearrange(
                                    "(c p) o -> p c o", p=128
                                ),
                            )
                    if tile_idx == 1:
                        nc.gpsimd.dma_start(
                            out=wdk, in_=wKd.rearrange("(c p) o -> p c o", p=128)
                        )
                        for bb in range(BL):
                            nc.gpsimd.dma_start(
                                out=OT[:, :, bb],
                                in_=optT[bb].rearrange("(c p) o l -> p c o l", p=128),
                            )
                    if tile_idx == 2:
                        nc.gpsimd.dma_start(
                            out=wqv, in_=wQV.rearrange("(c p) o -> p c o", p=128)
                        )
                    # natural-layout article for the PE weighted V-sum
                    An = nstream.tile([128, NLC, H], BF16, tag="nstream")
                    nc.sync.dma_start(
                        out=An,
                        in_=art16[b, ds(h * LH, LH), :].rearrange(
                            "(lc p) hh -> p lc hh", p=128
                        ),
                    )
                    mt = mtp.tile([128, C, LH], BF16, tag="mt")
                    for co in range(C):
                        if tile_idx == 0:
                            bias_a_block(co)
                        kp = pacc.tile([128, LH], F32, tag="acc")
                        for lt in range(2):
                            for cp in range(CP):
                                nc.tensor.matmul(
                                    kp[:, ds(lt * 512, 512)],
                                    lhsT=wk8[:, ds(2 * cp, 2), ts(co, 128)],
                                    rhs=T8[:, ds(2 * cp, 2), ds(lt * 512, 512)],
                                    start=(cp == 0),
                                    stop=(cp == CP - 1),
                                    perf_mode=DR,
                                )
                        nc.scalar.activation(
                            mt[:, co], kp, AF.Tanh, bias=biasA[:, co, b : b + 1]
                        )
                    # interleave the options K-projection while tanh drains
                    if tile_idx >= 1:
                        kpd_block(tile_idx - 1)
                    # score logits as PSUM columns; one closed group per lc
                    lgc = plgc.tile([128, NLC], F32, tag="lgc")
                    for lc in range(NLC):
                        for co in range(C):
                            nc.tensor.matmul(
                                lgc[:, lc : lc + 1],
                                lhsT=mt[:, co, ts(lc, 128)],
                                rhs=vwa[:, co : co + 1],
                                start=(co == 0),
                                stop=(co == C - 1),
                            )
                    sc = scp.tile([128, NLC], BF16, tag="sc")
                    nc.scalar.activation(sc, lgc, AF.Exp)
                    # weighted V-sum + softmax-sum on the PE; groups closed
                    # within this tile (columns h*C+co and 2C+h)
                    for co in range(C):
                        for lc in range(NLC):
                            nc.tensor.matmul(
                                pu[:, h * C + co : h * C + co + 1],
                                lhsT=An[:, lc, ts(co, 128)],
                                rhs=sc[:, lc : lc + 1],
                                start=(lc == 0),
                                stop=(lc == NLC - 1),
                            )
                    for lc in range(NLC):
                        nc.tensor.matmul(
                            pu[0:1, 2 * C + h : 2 * C + h + 1],
                            lhsT=ones128,
                            rhs=sc[:, lc : lc + 1],
                            start=(lc == 0),
                            stop=(lc == NLC - 1),
                        )
                    tile_idx += 1
                nc.vector.tensor_copy(uTun[:, :, b], pu[:, 0:C])
                nc.vector.tensor_add(uTun[:, :, b], uTun[:, :, b], pu[:, C : 2 * C])
                nc.vector.tensor_copy(ssums[:, b : b + 1], pu[0:1, 2 * C : 2 * C + 1])
                nc.vector.tensor_add(
                    ssums[:, b : b + 1],
                    ssums[:, b : b + 1],
                    pu[0:1, 2 * C + 1 : 2 * C + 2],
                )
            kpd_block(C - 1)

            # ---------- article normalization ----------
            rsb = one.tile([1, BL], F32, tag="rsb")
            nc.vector.reciprocal(rsb, ssums)
            rs_rep = one.tile([128, BL], F32, tag="rs_rep")
            nc.gpsimd.partition_broadcast(rs_rep, rsb)
            nc.vector.tensor_mul(
                uT, uTun, rs_rep.unsqueeze(1).broadcast_to((128, C, BL))
            )

            # ---------- option tanh bias via folded Wqv = d_Qw a_Vw^T ----------
            for co in range(C):
                pq2 = psml.tile([128, BL], F32, tag="sml")
                for ci in range(C):
                    nc.tensor.matmul(
                        pq2,
                        lhsT=wqv[:, ci, ts(co, 128)],
                        rhs=uT[:, ci, :],
                        start=(ci == 0),
                        stop=(ci == C - 1),
                    )
                nc.vector.tensor_scalar_add(biasO[:, co, :], pq2, qvb[:, co : co + 1])

            # ---------- options branch tail ----------
            # mdt = tanh(mdt_pre + biasO), pipelined in two b-halves
            mdt = scrp.tile([128, C, LH], BF16, tag="scr")
            HB = OL // 2  # 320 columns = 2 batch items
            lgd = pacc.tile([1, 2, 512], F32, tag="acc")  # bank-aligned halves
            for bh in range(2):
                sl = ds(bh * HB, HB)
                nc.vector.tensor_add(
                    mdt[:, :, sl].rearrange("p c (b ol) -> p c b ol", b=2),
                    mdt_pre[:, :, sl].rearrange("p c (b ol) -> p c b ol", b=2),
                    biasO[:, :, ds(bh * 2, 2)]
                    .unsqueeze(-1)
                    .broadcast_to((128, C, 2, NOPT * LO)),
                )
                nc.scalar.activation(mdt[:, :, sl], mdt[:, :, sl], AF.Tanh)
                for co in range(C):
                    nc.tensor.matmul(
                        lgd[:, bh, 0:HB],
                        lhsT=vwd[:, co : co + 1],
                        rhs=mdt[:, co, sl],
                        start=(co == 0),
                        stop=(co == C - 1),
                    )
            s_d = one.tile([1, OL], BF16, tag="s_d")
            nc.scalar.activation(
                s_d.rearrange("p (bh x) -> p bh x", bh=2), lgd[:, :, 0:HB], AF.Exp
            )
            sums_d = one.tile([1, BO, 1], F32, tag="sums_d")
            nc.vector.tensor_reduce(
                sums_d,
                s_d.rearrange("p (bo l) -> p bo l", l=LO),
                axis=AX.X,
                op=ALU.add,
            )
            rec_d = one.tile([1, BO], F32, tag="rec_d")
            nc.vector.reciprocal(rec_d, sums_d.rearrange("p bo one -> p (bo one)"))
            sdn = one.tile([1, OL], BF16, tag="sdn")
            nc.vector.tensor_mul(
                sdn.rearrange("p (bo l) -> p bo l", l=LO),
                s_d.rearrange("p (bo l) -> p bo l", l=LO),
                rec_d.unsqueeze(-1).broadcast_to((1, BO, LO)),
            )
            sdrep = srp.tile([128, LH], BF16, tag="srep")
            nc.gpsimd.partition_broadcast(sdrep[:, 0:OL], sdn)

            # normalized weighted V-sum over options + folded final linear
            scrd = scrp.tile([128, C, LH], BF16, tag="scr")
            OTf = OT.rearrange("p c b o l -> p c (b o l)")
            nc.vector.tensor_mul(
                scrd[:, :, 0:OL],
                OTf,
                sdrep[:, 0:OL].unsqueeze(1).broadcast_to((128, C, OL)),
            )
            u_d = one.tile([128, C, BO, 1], F32, tag="u_d")
            nc.vector.tensor_reduce(
                u_d,
                scrd[:, :, 0:OL].rearrange("p c (bo l) -> p c bo l", l=LO),
                axis=AX.X,
                op=ALU.add,
            )
            u16 = one.tile([128, C, BL, NOPT], BF16, tag="u16")
            nc.vector.tensor_copy(
                u16.rearrange("p c b o -> p c (b o)"),
                u_d.rearrange("p c bo one -> p c (bo one)"),
            )
            pout = psml.tile([BL, OUTP], F32, tag="sml")
            for c in range(C):
                for o in range(NOPT):
                    nc.tensor.matmul(
                        pout,
                        lhsT=u16[:, c, :, o],
                        rhs=fw[:, o, c, :],
                        start=(c == 0 and o == 0),
                        stop=(c == C - 1 and o == NOPT - 1),
                    )
            out_s = one.tile([BL, OUTP], F32, tag="out_s")
            nc.vector.tensor_add(out_s, pout, fb)
            nc.sync.dma_start(out=outd, in_=out_s[:, 0:OUT])

    nc.compile()
    return nc


@functools.lru_cache(maxsize=1)
def get_nc() -> bass.Bass:
    return build_nc()


def make_in_maps(inputs: dict) -> list[dict]:
    bf16 = ml_dtypes.bfloat16
    fp8 = ml_dtypes.float8_e4m3
    art = np.ascontiguousarray(np.asarray(inputs["article_contexts"], np.float32))
    qc = np.asarray(inputs["question_contexts"], np.float32)
    opt = np.ascontiguousarray(np.asarray(inputs["options_embeds"], np.float32))
    idx = np.asarray(inputs["answer_indices"]).astype(np.int64)

    def g(name):
        return np.asarray(inputs[name], np.float32)

    aQwT = np.ascontiguousarray(g("a_Qw").T).astype(bf16)
    aKwT = np.ascontiguousarray(g("a_Kw").T).astype(fp8)
    dKwT = np.ascontiguousarray(g("d_Kw").T).astype(bf16)
    # folded: aq -> options query projection
    Wqv = g("d_Qw") @ g("a_Vw")  # [H, H]
    qvwT = np.ascontiguousarray(Wqv.T).astype(bf16)
    bias_qv = g("d_Qw") @ g("a_Vb") + g("d_Qb") + g("d_Kb")  # [H]
    # folded: per-option final weights
    f_w = g("f_w")  # [OUT, 5H]
    dVwT = g("d_Vw").T
    Ff = np.stack(
        [dVwT @ f_w[:, o * H : (o + 1) * H].T for o in range(NOPT)], axis=0
    )  # [o, H_in, OUT]
    fb_new = g("f_b") + sum(
        f_w[:, o * H : (o + 1) * H] @ g("d_Vb") for o in range(NOPT)
    )  # [OUT]
    fwT = np.zeros((128, NOPT, C, OUTP), np.float32)
    fwT[:, :, :, :OUT] = Ff.reshape(NOPT, C, 128, OUT).transpose(2, 0, 1, 3)

    def colvec(v, dt):  # [H] -> [128, C] chunk-major
        return np.ascontiguousarray(
            np.asarray(v, np.float32).reshape(C, 128).T
        ).astype(dt)

    vwaT = colvec(g("a_vw").reshape(H), bf16)
    vwdT = colvec(g("d_vw").reshape(H), bf16)
    qkbT = colvec(g("a_Qb") + g("a_Kb"), np.float32)
    qvbT = colvec(bias_qv, np.float32)

    artT = np.ascontiguousarray(art.transpose(0, 2, 1)).astype(fp8)  # [B, H, LA]
    artN = art.astype(bf16)  # [B, LA, H] natural layout for the PE V-sum
    optT = np.ascontiguousarray(opt.transpose(0, 3, 1, 2)).astype(bf16)
    onehot = np.zeros((B, LQ), np.float32)
    onehot[np.arange(B), idx] = 1.0

    shared = dict(
        aQwT=aQwT, aKwT=aKwT, qvwT=qvwT, dKwT=dKwT,
        vwaT=vwaT, vwdT=vwdT, qkbT=qkbT, qvbT=qvbT,
        fwT=fwT.astype(bf16),
        fb=np.ascontiguousarray(
            np.tile(np.pad(fb_new.astype(np.float32), (0, 3)).reshape(1, 8), (BL, 1))
        ),
    )
    in_maps = []
    for r in range(NCORES):
        s = slice(r * BL, (r + 1) * BL)
        m = dict(shared)
        m["art8"] = artT[s]
        m["art16"] = artN[s]
        m["optT"] = optT[s]
        m["qc"] = qc[s].astype(bf16)
        m["oh"] = np.ascontiguousarray(onehot[s].T).astype(bf16)
        in_maps.append(m)
    return in_maps


def run(inputs: dict, trace: bool = False, tmpdir=None):
    from concourse.bass_utils import run_bass_kernel_spmd

    nc = get_nc()
    in_maps = make_in_maps(inputs)
    res = run_bass_kernel_spmd(
        nc, in_maps, core_ids=list(range(NCORES)), trace=trace, tmpdir=tmpdir
    )
    out = np.concatenate([res.results[r]["out"] for r in range(NCORES)], axis=0)
    return out, res


def kernel(**inputs) -> np.ndarray:
    out, _ = run(inputs, trace=False)
    return out
